# revision 33
# baseline (speedup 1.0000x reference)
"""ChannelSelfAttn Trainium2 kernel (bf16 I/O).

Reference computation (per sample b, x_b: [C=64, T=4000]):
    q = w1*x + b1, k = w2*x + b2 broadcast over F=16 feature maps
    e[i,j] = sum_{f,t} q[f,i,t]*k[f,j,t]
           = A*G[i,j] + B1*s_i + B2*s_j + C0*T
      where G = x_b @ x_b.T, s = rowsum(x_b),
            A = w1.w2, B1 = w1.b2, B2 = b1.w2, C0 = b1.b2
    e <- (e - min_j e)/(max_j e - min_j e + 1e-8)   # row terms B1*s_i, C0*T cancel
    e <- softmax_j(e)
    out = gamma * (e @ x_b) + x_b

Only f = A*G + B2*s_j survives the normalize. Furthermore
    f[i,j] = sum_t (A*xT[t,i] + B2) * xT[t,j]
so an affine copy of the transposed chunks acts as lhsT and the gram matmul
produces f directly -- no separate row-sum extraction / broadcast.
Softmax shift-invariance lets us drop the "-min" bias: exp(f*rr) with
rr = 1/(max-min+eps) gives identical normalized weights (args stay in
[-1.2, 1.2] because the Gram diagonal dominates).

I/O is bf16: the host quantizes x to bf16 (max abs err 2^-9*|x|, ~0.3% of the
output scale, vs the 2e-2 gate) and upconverts the bf16 result; this halves
HBM traffic, which bounds the fp32 version.

Sharding: data-parallel over batch. 32 samples / 8 cores = 4 samples/core,
processed as 2 pairs; each pair stacks 2 samples' channels into the 128
SBUF partitions.

Final schedule (build_program_v5): three-stage pipeline (load | gram | attn)
emitted with stage A transpose/gram groups and stage B attn/epi chunks
interleaved at 1024-col granularity, so each in-order engine queue alternates
between the two in-flight pairs instead of head-of-line blocking. Transpose
PSUM groups are 1024 cols wide (half the PSUM-drain instructions); x-tail and
gw zeroing happen once per physical buffer instead of per pair.

All DMAs ride the SP hardware queue, with input loads PREFETCHED two pipeline
steps ahead (x_bufs=5) so they sit in front of semaphore-waiting output DMAs
in the in-order queue — out-DMAs waiting at the queue head for epilogue sems
otherwise stall the next pair's loads behind them. Inputs stay at 4 pieces per
pair (arrival granularity gates transpose-group start; 1-3 pieces measured
slower), outputs are one 1MB contiguous store per pair (fewer queue entries
win monotonically once prefetch decouples them). Steady state ~14.6us/body =
11.4us DMA wire + ~3us queue overhead; PE 9.4us and DVE/ACT drains 9.8us sit
underneath. Alternate queues all measured worse in-kernel: ACT-queue DMAs
stall the ACT sequencer (sem waits block at the issuing SEQ, and for outputs
ACT itself produces the gating epilogue chunks); gpsimd SWDGE adds ~1us/DMA
of Q7 descriptor-gen; splitting inputs across SP+ACT loses 2.5us even with
stale WAR sems.
"""

import numpy as np
import ml_dtypes

import concourse.bacc as bacc
import concourse.bass as bass
import concourse.mybir as mybir
import concourse.tile as tile
from concourse.bass_utils import run_bass_kernel_spmd

FP32 = mybir.dt.float32
BF16 = mybir.dt.bfloat16
AF = mybir.ActivationFunctionType
ALU = mybir.AluOpType
AX = mybir.AxisListType

B, C, T = 32, 64, 4000
N_CORES = 8
SPC = B // N_CORES          # samples per core = 4
PAIRS = SPC // 2            # 2
TPAD = 4096                 # T padded to 32 chunks of 128
NCHUNK = TPAD // 128        # 32
NATT = TPAD // 512          # 8 attention N-chunks


def build_program_v2(A: float, B2: float, gamma: float, replicate: int = 1,
                     in_pieces: int = 4, out_pieces: int = 4,
                     epi_dve: int = 4, x3: bool = True,
                     out_on_act: bool = False, g_act_every: int = 0,
                     scalar_ap: bool = True, pa3: bool = True,
                     loops: int = 0) -> bass.Bass:
    """Single-tile augmented gram:
        g[t,i] = a*xT[t,i] + b   with a = sqrt(|A|) (folded into a scaled
                                 identity used by the PE transposes),
                                 b chosen so sum_t g_i*g_j = sigma*(A*G +
                                 B2*s_j) + row-const, sigma = sign(A)
        f'' = sum_k g_k.T @ g_k accumulates in PSUM; minmax-normalize+exp with
        scale sigma*rr recovers the exact softmax weights (row-consts cancel).
    The gamma/se normalization is prescaled into the weights and the residual
    +x is folded in as +I on the transposed weight matrix, so the attn
    epilogue is a pure PSUM->SBUF bf16 copy.
    """
    sigma = 1.0 if A >= 0 else -1.0
    a_eff = float(np.sqrt(abs(A)))
    b_eff = sigma * B2 / a_eff if A != 0 else 0.0

    nc = bacc.Bacc(None)
    x_h = nc.declare_dram_parameter("x", [SPC * C, T], BF16, isOutput=False)
    id_h = nc.declare_dram_parameter("ident", [128, 128], BF16, isOutput=False)
    out_h = nc.declare_dram_parameter("out", [SPC * C, T], BF16, isOutput=True)

    with tile.TileContext(nc) as tc:
        with (
            tc.tile_pool(name="xio", bufs=3 if x3 else 2) as p_x,
            tc.tile_pool(name="g", bufs=2) as p_g,
            tc.tile_pool(name="outb", bufs=2) as p_out,
            tc.tile_pool(name="small", bufs=2) as p_small,
            tc.tile_pool(name="const", bufs=1) as p_const,
            tc.tile_pool(name="pt", bufs=2, space="PSUM") as p_pt,
            tc.tile_pool(name="pg", bufs=2, space="PSUM") as p_pg,
            tc.tile_pool(name="pa", bufs=3 if pa3 else 2, space="PSUM") as p_pa,
            tc.tile_pool(name="ps", bufs=1 if pa3 else 2, space="PSUM") as p_ps,
        ):
            ident1 = p_const.tile([128, 128], BF16)
            nc.sync.dma_start(ident1[:], id_h[:, :])
            # warm the ACT function table before the critical tail needs Exp
            dumm = p_const.tile([1, 1], FP32)
            nc.scalar.activation(dumm[:], ident1[0:1, 0:1], AF.Exp)
            if scalar_ap:
                a_ap = p_const.tile([128, 1], FP32)
                nc.vector.memset(a_ap[:], float(a_eff))
                b_ap = p_const.tile([128, 1], FP32)
                nc.vector.memset(b_ap[:], float(b_eff))

            all_pairs = [pp for _ in range(replicate) for pp in range(PAIRS)]

            def emit_in(p, idx):
                rows = slice(idx * 128, (idx + 1) * 128)
                x_bf = p_x.tile([128, TPAD], BF16)
                ipc = T // in_pieces
                for i in range(in_pieces):
                    c0, c1 = i * ipc, (i + 1) * ipc
                    nc.sync.dma_start(x_bf[:, c0:c1], x_h[rows, c0:c1])
                nc.vector.memset(x_bf[:, T:TPAD], 0.0)
                return x_bf

            def emit_stage_a(x_bf, idx):
                # gw zeroed early so the tail only fills the diagonal blocks
                gw = p_small.tile([128, 128], BF16, tag="gw")
                nc.vector.memset(gw[:], 0.0)

                # transposes (PE, scaled identity -> pt = a*xT) interleaved
                # with the gram accumulation, one group behind
                g = p_g.tile([128, TPAD], BF16)
                pg = p_pg.tile([128, 128], FP32, tag="pg")

                def emit_trans(q):
                    pt = p_pt.tile([128, 512], BF16, tag="pt")
                    for j in range(4):
                        k = 4 * q + j
                        nc.tensor.transpose(
                            pt[:, j * 128:(j + 1) * 128],
                            x_bf[:, k * 128:(k + 1) * 128],
                            ident1[:, :],
                        )
                    gq = g[:, q * 512:(q + 1) * 512]
                    if g_act_every and q % g_act_every == g_act_every - 1:
                        nc.scalar.activation(gq, pt[:], AF.Copy,
                                             bias=float(b_eff), scale=float(a_eff))
                    elif scalar_ap:
                        nc.vector.tensor_scalar(gq, pt[:], a_ap[:], b_ap[:],
                                                op0=ALU.mult, op1=ALU.add)
                    else:
                        nc.vector.tensor_scalar(gq, pt[:], float(a_eff),
                                                float(b_eff),
                                                op0=ALU.mult, op1=ALU.add)

                def emit_gram(q):
                    for j in range(4):
                        k = 4 * q + j
                        nc.tensor.matmul(
                            pg[:], lhsT=g[:, k * 128:(k + 1) * 128],
                            rhs=g[:, k * 128:(k + 1) * 128],
                            start=(k == 0), stop=(k == NCHUNK - 1),
                        )

                emit_trans(0)
                for q in range(1, NCHUNK // 4):
                    emit_trans(q)
                    emit_gram(q - 1)
                emit_gram(NCHUNK // 4 - 1)

                # ---- row minmax + exp on the diagonal blocks (from PSUM)
                mx = p_small.tile([128, 1], FP32, tag="mx")
                nc.vector.reduce_max(mx[0:64], pg[0:64, 0:64], axis=AX.X)
                nc.vector.reduce_max(mx[64:128], pg[64:128, 64:128], axis=AX.X)
                mn = p_small.tile([128, 1], FP32, tag="mn")
                nc.vector.tensor_reduce(mn[0:64], pg[0:64, 0:64], axis=AX.X, op=ALU.min)
                nc.vector.tensor_reduce(mn[64:128], pg[64:128, 64:128], axis=AX.X, op=ALU.min)
                dd = p_small.tile([128, 1], FP32, tag="dd")
                nc.vector.scalar_tensor_tensor(
                    dd[:], mx[:], 1e-8, mn[:], op0=ALU.add, op1=ALU.subtract,
                )
                rr = p_small.tile([128, 1], FP32, tag="rr")
                nc.vector.reciprocal(rr[:], dd[:])
                if sigma < 0:
                    nc.vector.tensor_scalar_mul(rr[:], rr[:], -1.0)
                se = p_small.tile([128, 1], FP32, tag="se")
                nc.scalar.activation(
                    gw[0:64, 0:64], pg[0:64, 0:64], AF.Exp,
                    scale=rr[0:64], accum_out=se[0:64],
                )
                nc.scalar.activation(
                    gw[64:128, 64:128], pg[64:128, 64:128], AF.Exp,
                    scale=rr[64:128], accum_out=se[64:128],
                )
                return x_bf, idx, gw, se

            def emit_stage_b(x_bf, idx, gw, se):
                rows = slice(idx * 128, (idx + 1) * 128)
                rs = p_small.tile([128, 1], FP32, tag="rs")
                nc.vector.reciprocal(rs[:], se[:])
                wsc = p_small.tile([128, 1], FP32, tag="wsc")
                nc.vector.tensor_scalar_mul(wsc[:], rs[:], float(gamma))
                gws = p_small.tile([128, 128], BF16, tag="gws")
                nc.vector.tensor_scalar(gws[:], gw[:], wsc[:], None, op0=ALU.mult)

                # ---- blk2 = (gws)^T + I so attn matmul includes +x
                pb = p_ps.tile([128, 128], BF16, tag="ps")
                nc.tensor.transpose(pb[:], gws[:], ident1[:, :])
                blk2 = p_small.tile([128, 128], BF16, tag="blk")
                nc.vector.tensor_add(blk2[:], pb[:], ident1[:])

                # ---- out = blk2.T @ x ; epilogue is a pure bf16 copy
                ob = p_out.tile([128, TPAD], BF16)
                for n in range(NATT):
                    pa = p_pa.tile([128, 512], FP32, tag="pa")
                    nc.tensor.matmul(pa[:], lhsT=blk2[:],
                                     rhs=x_bf[:, n * 512:(n + 1) * 512],
                                     start=True, stop=True)
                    obc = ob[:, n * 512:(n + 1) * 512]
                    if (n * epi_dve) % NATT < epi_dve:
                        nc.vector.tensor_copy(obc, pa[:])
                    else:
                        nc.scalar.copy(obc, pa[:])

                opc = T // out_pieces
                for i in range(out_pieces):
                    c0, c1 = i * opc, (i + 1) * opc
                    eng = nc.scalar if out_on_act else nc.sync
                    eng.dma_start(out_h[rows, c0:c1], ob[:, c0:c1])

            # software pipeline, depth 3: in(p+2) | stageA(p+1) | stageB(p)
            def emit_pipeline():
                xq, aq = [], []
                for p, idx in enumerate(all_pairs):
                    xq.append((emit_in(p, idx), idx))
                    if len(xq) > 1:
                        xb, i0 = xq.pop(0)
                        aq.append(emit_stage_a(xb, i0))
                    if len(aq) > 1:
                        emit_stage_b(*aq.pop(0))
                for xb, i0 in xq:
                    aq.append(emit_stage_a(xb, i0))
                for a in aq:
                    emit_stage_b(*a)

            if loops:
                with tc.For_i(0, loops, 1):
                    emit_pipeline()
            else:
                emit_pipeline()

    nc.finalize()
    return nc


def build_program_bf16(A: float, B2: float, gamma: float, replicate: int = 1,
                       in_pieces: int = 4, out_pieces: int = 8,
                       out_on_act: bool = True, in_split: bool = False,
                       x3: bool = True, xt3: bool = False) -> bass.Bass:
    nc = bacc.Bacc(None)
    x_h = nc.declare_dram_parameter("x", [SPC * C, T], BF16, isOutput=False)
    id_h = nc.declare_dram_parameter("ident", [128, 128], BF16, isOutput=False)
    out_h = nc.declare_dram_parameter("out", [SPC * C, T], BF16, isOutput=True)

    with tile.TileContext(nc) as tc:
        with (
            tc.tile_pool(name="xio", bufs=3 if x3 else 2) as p_x,
            tc.tile_pool(name="xT", bufs=3 if xt3 else 2) as p_xT,
            tc.tile_pool(name="xTa", bufs=3 if xt3 else 2) as p_xTa,
            tc.tile_pool(name="outb", bufs=2) as p_out,
            tc.tile_pool(name="small", bufs=2) as p_small,
            tc.tile_pool(name="const", bufs=1) as p_const,
            tc.tile_pool(name="pt", bufs=2, space="PSUM") as p_pt,
            tc.tile_pool(name="pg", bufs=2, space="PSUM") as p_pg,
            tc.tile_pool(name="pa", bufs=2, space="PSUM") as p_pa,
            tc.tile_pool(name="ps", bufs=2, space="PSUM") as p_ps,
        ):
            ident = p_const.tile([128, 128], BF16)
            nc.sync.dma_start(ident[:], id_h[:, :])

            for p in [pp for _ in range(replicate) for pp in range(PAIRS)]:
                rows = slice(p * 128, (p + 1) * 128)

                # ---- load x pair [128, 4000] bf16, zero-pad t to 4096
                x_bf = p_x.tile([128, TPAD], BF16)
                ipc = T // in_pieces
                for i in range(in_pieces):
                    c0, c1 = i * ipc, (i + 1) * ipc
                    eng = nc.scalar if (in_split and i % 2) else nc.sync
                    eng.dma_start(x_bf[:, c0:c1], x_h[rows, c0:c1])
                nc.vector.memset(x_bf[:, T:TPAD], 0.0)

                # ---- transpose to xT [t, c] chunks via PE; also write the
                # affine copy xTa = A*xT + B2 that serves as the gram lhsT
                xT = p_xT.tile([128, TPAD], BF16)
                xTa = p_xTa.tile([128, TPAD], BF16)
                for q in range(NCHUNK // 4):
                    pt = p_pt.tile([128, 512], BF16, tag="pt")
                    for j in range(4):
                        k = 4 * q + j
                        nc.tensor.transpose(
                            pt[:, j * 128:(j + 1) * 128],
                            x_bf[:, k * 128:(k + 1) * 128],
                            ident[:, :],
                        )
                    dst_p = xT[:, q * 512:(q + 1) * 512]
                    dst_a = xTa[:, q * 512:(q + 1) * 512]
                    if q % 2 == 0:
                        nc.vector.tensor_copy(dst_p, pt[:])
                        nc.scalar.activation(dst_a, pt[:], AF.Copy,
                                             bias=float(B2), scale=float(A))
                    else:
                        nc.scalar.copy(dst_p, pt[:])
                        nc.vector.tensor_scalar(dst_a, pt[:], float(A), float(B2),
                                                op0=ALU.mult, op1=ALU.add)

                # ---- gram: pg = sum_k xTa_k.T @ xT_k = A*G + B2*s_j (+ garbage
                # off-diagonal cross-sample blocks, ignored)
                pg = p_pg.tile([128, 128], FP32, tag="pg")
                for k in range(NCHUNK):
                    nc.tensor.matmul(
                        pg[:], lhsT=xTa[:, k * 128:(k + 1) * 128],
                        rhs=xT[:, k * 128:(k + 1) * 128],
                        start=(k == 0), stop=(k == NCHUNK - 1),
                    )

                # ---- row minmax-normalize + exp (+ rowsum for softmax denom),
                # reading the diagonal blocks of pg straight from PSUM
                mx = p_small.tile([128, 1], FP32, tag="mx")
                nc.vector.reduce_max(mx[0:64], pg[0:64, 0:64], axis=AX.X)
                nc.vector.reduce_max(mx[64:128], pg[64:128, 64:128], axis=AX.X)
                mn = p_small.tile([128, 1], FP32, tag="mn")
                nc.vector.tensor_reduce(mn[0:64], pg[0:64, 0:64], axis=AX.X, op=ALU.min)
                nc.vector.tensor_reduce(mn[64:128], pg[64:128, 64:128], axis=AX.X, op=ALU.min)
                dd = p_small.tile([128, 1], FP32, tag="dd")
                nc.vector.scalar_tensor_tensor(
                    dd[:], mx[:], 1e-8, mn[:], op0=ALU.add, op1=ALU.subtract,
                )
                rr = p_small.tile([128, 1], FP32, tag="rr")
                nc.vector.reciprocal(rr[:], dd[:])
                # exp(f*rr) into diagonal blocks of a zeroed [128,128] tile;
                # the dropped -mn*rr bias is a per-row constant that cancels in
                # the softmax normalization (se accumulates the same factor).
                gw = p_small.tile([128, 128], BF16, tag="gw")
                nc.vector.memset(gw[:], 0.0)
                se = p_small.tile([128, 1], FP32, tag="se")
                nc.scalar.activation(
                    gw[0:64, 0:64], pg[0:64, 0:64], AF.Exp,
                    scale=rr[0:64], accum_out=se[0:64],
                )
                nc.scalar.activation(
                    gw[64:128, 64:128], pg[64:128, 64:128], AF.Exp,
                    scale=rr[64:128], accum_out=se[64:128],
                )
                rs = p_small.tile([128, 1], FP32, tag="rs")
                nc.vector.reciprocal(rs[:], se[:])
                wsc = p_small.tile([128, 1], FP32, tag="wsc")
                nc.vector.tensor_scalar_mul(wsc[:], rs[:], float(gamma))

                # ---- block-diag transposed weights for attn matmul
                pb = p_ps.tile([128, 128], BF16, tag="ps")
                nc.tensor.transpose(pb[:], gw[:], ident[:, :])
                blk = p_small.tile([128, 128], BF16, tag="blk")
                nc.vector.tensor_copy(blk[:], pb[:])

                # ---- attn = blk.T @ x (unnormalized), out = wsc*attn + x
                ob = p_out.tile([128, TPAD], BF16)
                for n in range(NATT):
                    pa = p_pa.tile([128, 512], FP32, tag="pa")
                    xc = x_bf[:, n * 512:(n + 1) * 512]
                    nc.tensor.matmul(pa[:], lhsT=blk[:], rhs=xc,
                                     start=True, stop=True)
                    obc = ob[:, n * 512:(n + 1) * 512]
                    if n % 2 == 1:
                        nc.scalar.mul(obc, pa[:], wsc[:])
                        nc.vector.tensor_add(obc, obc, xc)
                    else:
                        nc.vector.scalar_tensor_tensor(
                            obc, pa[:], wsc[:], xc, op0=ALU.mult, op1=ALU.add,
                        )

                opc = T // out_pieces
                for i in range(out_pieces):
                    c0, c1 = i * opc, (i + 1) * opc
                    eng = nc.scalar if out_on_act else nc.sync
                    eng.dma_start(out_h[rows, c0:c1], ob[:, c0:c1])

    nc.finalize()
    return nc


def build_program_v4(A: float, B2: float, gamma: float, replicate: int = 1,
                     in_pieces: int = 4, out_pieces: int = 4,
                     wide_pt: bool = True, g_dve: str = "vava",
                     epi_dve: str = "vavavava", pool_small: bool = True,
                     once_init: bool = True, x_bufs: int = 3,
                     out_eng: str = "s", in_eng: str = "s",
                     pa_bufs: int = 3, loops: int = 0) -> bass.Bass:
    """v2 + structural cleanups:
    - transpose PSUM groups widened to 1024 cols (half the drain instrs)
    - x-tail / gw zeroing hoisted out of the steady-state loop (done once per
      physical buffer, on the otherwise-idle Pool engine)
    - gws scaling moved to Pool (SBUF-only op)
    - per-chunk drain engine assignment via pattern strings ('v'=DVE, 'a'=ACT)
    - DMA piece counts tunable; out DMAs optionally on the ACT HWDGE queue
      ('a') or Pool SWDGE ('p')
    """
    sigma = 1.0 if A >= 0 else -1.0
    a_eff = float(np.sqrt(abs(A)))
    b_eff = sigma * B2 / a_eff if A != 0 else 0.0

    nc = bacc.Bacc(None)
    x_h = nc.declare_dram_parameter("x", [SPC * C, T], BF16, isOutput=False)
    id_h = nc.declare_dram_parameter("ident", [128, 128], BF16, isOutput=False)
    out_h = nc.declare_dram_parameter("out", [SPC * C, T], BF16, isOutput=True)

    eng_of = {"v": None, "a": None}  # filled after nc exists

    def dma_eng(code):
        return {"s": nc.sync, "a": nc.scalar, "p": nc.gpsimd}[code]

    GW = 1024 if wide_pt else 512          # transpose group width
    NGR = TPAD // GW                       # transpose groups per pair
    TPG = GW // 128                        # transposes per group

    with tile.TileContext(nc) as tc:
        with (
            tc.tile_pool(name="xio", bufs=x_bufs) as p_x,
            tc.tile_pool(name="g", bufs=2) as p_g,
            tc.tile_pool(name="outb", bufs=2) as p_out,
            tc.tile_pool(name="small", bufs=2) as p_small,
            tc.tile_pool(name="const", bufs=1) as p_const,
            tc.tile_pool(name="pt", bufs=2, space="PSUM") as p_pt,
            tc.tile_pool(name="pg", bufs=2, space="PSUM") as p_pg,
            tc.tile_pool(name="pa", bufs=pa_bufs, space="PSUM") as p_pa,
            tc.tile_pool(name="ps", bufs=1, space="PSUM") as p_ps,
        ):
            ident1 = p_const.tile([128, 128], BF16)
            nc.sync.dma_start(ident1[:], id_h[:, :])
            dumm = p_const.tile([1, 1], FP32)
            nc.scalar.activation(dumm[:], ident1[0:1, 0:1], AF.Exp)
            a_ap = p_const.tile([128, 1], FP32)
            nc.vector.memset(a_ap[:], float(a_eff))
            b_ap = p_const.tile([128, 1], FP32)
            nc.vector.memset(b_ap[:], float(b_eff))

            all_pairs = [pp for _ in range(replicate) for pp in range(PAIRS)]
            n_x_alloc = [0]
            n_gw_alloc = [0]

            def emit_in(p, idx):
                rows = slice(idx * 128, (idx + 1) * 128)
                x_bf = p_x.tile([128, TPAD], BF16)
                if once_init and n_x_alloc[0] < x_bufs:
                    n_x_alloc[0] += 1
                    eng = nc.gpsimd if pool_small else nc.vector
                    eng.memset(x_bf[:, T:TPAD], 0.0)
                elif not once_init:
                    nc.vector.memset(x_bf[:, T:TPAD], 0.0)
                ipc = T // in_pieces
                for i in range(in_pieces):
                    c0, c1 = i * ipc, (i + 1) * ipc
                    dma_eng(in_eng).dma_start(x_bf[:, c0:c1], x_h[rows, c0:c1])
                return x_bf

            def emit_stage_a(x_bf, idx):
                gw = p_small.tile([128, 128], BF16, tag="gw")
                if once_init and n_gw_alloc[0] < 2:
                    n_gw_alloc[0] += 1
                    (nc.gpsimd if pool_small else nc.vector).memset(gw[:], 0.0)
                elif not once_init:
                    nc.vector.memset(gw[:], 0.0)

                g = p_g.tile([128, TPAD], BF16)
                pg = p_pg.tile([128, 128], FP32, tag="pg")

                def emit_trans(q):
                    pt = p_pt.tile([128, GW], BF16, tag="pt")
                    for j in range(TPG):
                        k = TPG * q + j
                        nc.tensor.transpose(
                            pt[:, j * 128:(j + 1) * 128],
                            x_bf[:, k * 128:(k + 1) * 128],
                            ident1[:, :],
                        )
                    gq = g[:, q * GW:(q + 1) * GW]
                    eng = nc.vector if g_dve[q % len(g_dve)] == "v" else nc.scalar
                    if eng is nc.vector:
                        eng.tensor_scalar(gq, pt[:], a_ap[:], b_ap[:],
                                          op0=ALU.mult, op1=ALU.add)
                    else:
                        nc.scalar.activation(gq, pt[:], AF.Copy,
                                             bias=float(b_eff), scale=float(a_eff))

                def emit_gram(q):
                    for j in range(TPG):
                        k = TPG * q + j
                        nc.tensor.matmul(
                            pg[:], lhsT=g[:, k * 128:(k + 1) * 128],
                            rhs=g[:, k * 128:(k + 1) * 128],
                            start=(k == 0), stop=(k == NCHUNK - 1),
                        )

                emit_trans(0)
                for q in range(1, NGR):
                    emit_trans(q)
                    emit_gram(q - 1)
                emit_gram(NGR - 1)

                mx = p_small.tile([128, 1], FP32, tag="mx")
                nc.vector.reduce_max(mx[0:64], pg[0:64, 0:64], axis=AX.X)
                nc.vector.reduce_max(mx[64:128], pg[64:128, 64:128], axis=AX.X)
                mn = p_small.tile([128, 1], FP32, tag="mn")
                nc.vector.tensor_reduce(mn[0:64], pg[0:64, 0:64], axis=AX.X, op=ALU.min)
                nc.vector.tensor_reduce(mn[64:128], pg[64:128, 64:128], axis=AX.X, op=ALU.min)
                dd = p_small.tile([128, 1], FP32, tag="dd")
                nc.vector.scalar_tensor_tensor(
                    dd[:], mx[:], 1e-8, mn[:], op0=ALU.add, op1=ALU.subtract,
                )
                rr = p_small.tile([128, 1], FP32, tag="rr")
                nc.vector.reciprocal(rr[:], dd[:])
                if sigma < 0:
                    nc.vector.tensor_scalar_mul(rr[:], rr[:], -1.0)
                se = p_small.tile([128, 1], FP32, tag="se")
                nc.scalar.activation(
                    gw[0:64, 0:64], pg[0:64, 0:64], AF.Exp,
                    scale=rr[0:64], accum_out=se[0:64],
                )
                nc.scalar.activation(
                    gw[64:128, 64:128], pg[64:128, 64:128], AF.Exp,
                    scale=rr[64:128], accum_out=se[64:128],
                )
                return x_bf, idx, gw, se

            def emit_stage_b(x_bf, idx, gw, se):
                rows = slice(idx * 128, (idx + 1) * 128)
                rs = p_small.tile([128, 1], FP32, tag="rs")
                nc.vector.reciprocal(rs[:], se[:])
                wsc = p_small.tile([128, 1], FP32, tag="wsc")
                nc.vector.tensor_scalar_mul(wsc[:], rs[:], float(gamma))
                gws = p_small.tile([128, 128], BF16, tag="gws")
                eng = nc.gpsimd if pool_small else nc.vector
                eng.tensor_scalar(gws[:], gw[:], wsc[:], None, op0=ALU.mult)

                pb = p_ps.tile([128, 128], BF16, tag="ps")
                nc.tensor.transpose(pb[:], gws[:], ident1[:, :])
                blk2 = p_small.tile([128, 128], BF16, tag="blk")
                nc.vector.tensor_add(blk2[:], pb[:], ident1[:])

                ob = p_out.tile([128, TPAD], BF16)
                for n in range(NATT):
                    pa = p_pa.tile([128, 512], FP32, tag="pa")
                    nc.tensor.matmul(pa[:], lhsT=blk2[:],
                                     rhs=x_bf[:, n * 512:(n + 1) * 512],
                                     start=True, stop=True)
                    obc = ob[:, n * 512:(n + 1) * 512]
                    if epi_dve[n % len(epi_dve)] == "v":
                        nc.vector.tensor_copy(obc, pa[:])
                    else:
                        nc.scalar.copy(obc, pa[:])

                opc = T // out_pieces
                for i in range(out_pieces):
                    c0, c1 = i * opc, (i + 1) * opc
                    dma_eng(out_eng).dma_start(out_h[rows, c0:c1], ob[:, c0:c1])

            def emit_pipeline():
                xq, aq = [], []
                for p, idx in enumerate(all_pairs):
                    xq.append((emit_in(p, idx), idx))
                    if len(xq) > 1:
                        xb, i0 = xq.pop(0)
                        aq.append(emit_stage_a(xb, i0))
                    if len(aq) > 1:
                        emit_stage_b(*aq.pop(0))
                for xb, i0 in xq:
                    aq.append(emit_stage_a(xb, i0))
                for a in aq:
                    emit_stage_b(*a)

            if loops:
                with tc.For_i(0, loops, 1):
                    emit_pipeline()
            else:
                emit_pipeline()

    nc.finalize()
    return nc


def build_program_v5(A: float, B2: float, gamma: float, replicate: int = 1,
                     in_pieces: int = 4, out_pieces: int = 4,
                     g_dve: str = "vava", epi_dve: str = "vavavava",
                     pool_small: bool = False, x_bufs: int = 3,
                     pa_bufs: int = 3, out_eng: str = "s", in_eng: str = "s",
                     contig: bool = False, prefetch: bool = False,
                     ob_bufs: int = 2, pa_wide: bool = False,
                     pdist: int = 1, b_first: bool = False,
                     loops: int = 0) -> bass.Bass:
    """v4 + fine-grained interleaved emission: stage A(p+1) transpose/gram
    groups and stage B(p) attn/epi chunks are emitted alternately so each
    engine's in-order queue ping-pongs between the two pairs' work instead of
    head-of-line blocking a ready stage behind a stalled one. Input DMA pieces
    are 1024-aligned so transpose group q waits only on pieces <= q.
    """
    sigma = 1.0 if A >= 0 else -1.0
    a_eff = float(np.sqrt(abs(A)))
    b_eff = sigma * B2 / a_eff if A != 0 else 0.0

    nc = bacc.Bacc(None)
    if contig:
        # piece-contiguous DRAM layout (host pre/post-shuffles): each DMA
        # reads/writes one fully sequential HBM block
        x_h = nc.declare_dram_parameter(
            "x", [PAIRS * in_pieces, 128, T // in_pieces], BF16, isOutput=False)
        out_h = nc.declare_dram_parameter(
            "out", [PAIRS * out_pieces, 128, T // out_pieces], BF16,
            isOutput=True)
    else:
        x_h = nc.declare_dram_parameter("x", [SPC * C, T], BF16, isOutput=False)
        out_h = nc.declare_dram_parameter("out", [SPC * C, T], BF16,
                                          isOutput=True)
    id_h = nc.declare_dram_parameter("ident", [128, 128], BF16, isOutput=False)

    GW = 1024
    NGR = TPAD // GW          # 4 transpose groups
    TPG = GW // 128           # 8 transposes per group

    def dma_eng(code):
        return {"s": nc.sync, "a": nc.scalar, "p": nc.gpsimd}[code]

    with tile.TileContext(nc) as tc:
        with (
            tc.tile_pool(name="xio", bufs=x_bufs) as p_x,
            tc.tile_pool(name="g", bufs=2) as p_g,
            tc.tile_pool(name="outb", bufs=ob_bufs) as p_out,
            tc.tile_pool(name="small", bufs=2) as p_small,
            tc.tile_pool(name="const", bufs=1) as p_const,
            tc.tile_pool(name="pt", bufs=2, space="PSUM") as p_pt,
            tc.tile_pool(name="pg", bufs=2, space="PSUM") as p_pg,
            tc.tile_pool(name="pa", bufs=2 if pa_wide else pa_bufs,
                         space="PSUM") as p_pa,
            tc.tile_pool(name="ps", bufs=1, space="PSUM") as p_ps,
        ):
            ident1 = p_const.tile([128, 128], BF16)
            nc.sync.dma_start(ident1[:], id_h[:, :])
            dumm = p_const.tile([1, 1], FP32)
            nc.scalar.activation(dumm[:], ident1[0:1, 0:1], AF.Exp)
            a_ap = p_const.tile([128, 1], FP32)
            nc.vector.memset(a_ap[:], float(a_eff))
            b_ap = p_const.tile([128, 1], FP32)
            nc.vector.memset(b_ap[:], float(b_eff))

            all_pairs = [pp for _ in range(replicate) for pp in range(PAIRS)]
            n = len(all_pairs)
            n_x_alloc = [0]
            n_gw_alloc = [0]
            pool_eng = nc.gpsimd if pool_small else nc.vector

            # out piece i may be issued after epi chunk out_gate[i]
            opc = T // out_pieces
            out_gate = [max(0, -(-((i + 1) * opc) // 512) - 1)
                        for i in range(out_pieces)]

            xs, As, Bs = {}, {}, {}

            def f_in(p):
                idx = all_pairs[p]
                rows = slice(idx * 128, (idx + 1) * 128)
                x_bf = p_x.tile([128, TPAD], BF16)
                if n_x_alloc[0] < x_bufs:
                    n_x_alloc[0] += 1
                    pool_eng.memset(x_bf[:, T:TPAD], 0.0)
                if contig:
                    ipc = T // in_pieces
                    for i in range(in_pieces):
                        dma_eng(in_eng[i % len(in_eng)]).dma_start(
                            x_bf[:, i * ipc:(i + 1) * ipc],
                            x_h[idx * in_pieces + i])
                else:
                    bounds = [min(i * (TPAD // in_pieces), T)
                              for i in range(in_pieces + 1)]
                    for i in range(in_pieces):
                        c0, c1 = bounds[i], bounds[i + 1]
                        if c1 > c0:
                            dma_eng(in_eng[i % len(in_eng)]).dma_start(
                                x_bf[:, c0:c1], x_h[rows, c0:c1])
                xs[p] = x_bf

            def a_pre(p):
                gw = p_small.tile([128, 128], BF16, tag="gw")
                if n_gw_alloc[0] < 2:
                    n_gw_alloc[0] += 1
                    pool_eng.memset(gw[:], 0.0)
                g = p_g.tile([128, TPAD], BF16)
                pg = p_pg.tile([128, 128], FP32, tag="pg")
                As[p] = [xs[p], gw, g, pg]

            def a_group(p, q):
                x_bf, gw, g, pg = As[p]
                pt = p_pt.tile([128, GW], BF16, tag="pt")
                for j in range(TPG):
                    k = TPG * q + j
                    nc.tensor.transpose(
                        pt[:, j * 128:(j + 1) * 128],
                        x_bf[:, k * 128:(k + 1) * 128],
                        ident1[:, :],
                    )
                gq = g[:, q * GW:(q + 1) * GW]
                if g_dve[q % len(g_dve)] == "v":
                    nc.vector.tensor_scalar(gq, pt[:], a_ap[:], b_ap[:],
                                            op0=ALU.mult, op1=ALU.add)
                else:
                    nc.scalar.activation(gq, pt[:], AF.Copy,
                                         bias=float(b_eff), scale=float(a_eff))

            def a_gram(p, q):
                x_bf, gw, g, pg = As[p]
                for j in range(TPG):
                    k = TPG * q + j
                    nc.tensor.matmul(
                        pg[:], lhsT=g[:, k * 128:(k + 1) * 128],
                        rhs=g[:, k * 128:(k + 1) * 128],
                        start=(k == 0), stop=(k == NCHUNK - 1),
                    )

            def a_post(p):
                x_bf, gw, g, pg = As[p]
                mx = p_small.tile([128, 1], FP32, tag="mx")
                nc.vector.reduce_max(mx[0:64], pg[0:64, 0:64], axis=AX.X)
                nc.vector.reduce_max(mx[64:128], pg[64:128, 64:128], axis=AX.X)
                mn = p_small.tile([128, 1], FP32, tag="mn")
                nc.vector.tensor_reduce(mn[0:64], pg[0:64, 0:64], axis=AX.X,
                                        op=ALU.min)
                nc.vector.tensor_reduce(mn[64:128], pg[64:128, 64:128],
                                        axis=AX.X, op=ALU.min)
                dd = p_small.tile([128, 1], FP32, tag="dd")
                nc.vector.scalar_tensor_tensor(
                    dd[:], mx[:], 1e-8, mn[:], op0=ALU.add, op1=ALU.subtract,
                )
                rr = p_small.tile([128, 1], FP32, tag="rr")
                nc.vector.reciprocal(rr[:], dd[:])
                if sigma < 0:
                    nc.vector.tensor_scalar_mul(rr[:], rr[:], -1.0)
                se = p_small.tile([128, 1], FP32, tag="se")
                nc.scalar.activation(
                    gw[0:64, 0:64], pg[0:64, 0:64], AF.Exp,
                    scale=rr[0:64], accum_out=se[0:64],
                )
                nc.scalar.activation(
                    gw[64:128, 64:128], pg[64:128, 64:128], AF.Exp,
                    scale=rr[64:128], accum_out=se[64:128],
                )
                Bs[p] = [x_bf, all_pairs[p], gw, se]

            def b_pre(p):
                x_bf, idx, gw, se = Bs[p]
                rs = p_small.tile([128, 1], FP32, tag="rs")
                nc.vector.reciprocal(rs[:], se[:])
                wsc = p_small.tile([128, 1], FP32, tag="wsc")
                nc.vector.tensor_scalar_mul(wsc[:], rs[:], float(gamma))
                gws = p_small.tile([128, 128], BF16, tag="gws")
                pool_eng.tensor_scalar(gws[:], gw[:], wsc[:], None, op0=ALU.mult)
                if pa_wide:
                    pb = p_pa.tile([128, 128], BF16, tag="pa")
                else:
                    pb = p_ps.tile([128, 128], BF16, tag="ps")
                nc.tensor.transpose(pb[:], gws[:], ident1[:, :])
                blk2 = p_small.tile([128, 128], BF16, tag="blk")
                nc.vector.tensor_add(blk2[:], pb[:], ident1[:])
                ob = p_out.tile([128, TPAD], BF16)
                Bs[p] = [x_bf, idx, blk2, ob]

            def b_chunk(p, nk):
                x_bf, idx, blk2, ob = Bs[p][:4]
                if pa_wide:
                    # nk indexes 512-col halves; emit on odd halves only, as a
                    # [128,1024] 2-bank psum group with a single wide drain
                    if nk % 2 == 0:
                        pa = p_pa.tile([128, 1024], FP32, tag="pa")
                        Bs[p] = [x_bf, idx, blk2, ob, pa]
                        nc.tensor.matmul(pa[:, 0:512], lhsT=blk2[:],
                                         rhs=x_bf[:, nk * 512:(nk + 1) * 512],
                                         start=True, stop=True)
                        return
                    pa = Bs[p][4]
                    Bs[p] = [x_bf, idx, blk2, ob]
                    nc.tensor.matmul(pa[:, 512:1024], lhsT=blk2[:],
                                     rhs=x_bf[:, nk * 512:(nk + 1) * 512],
                                     start=True, stop=True)
                    q = nk // 2
                    obc = ob[:, q * 1024:(q + 1) * 1024]
                    if epi_dve[q % len(epi_dve)] == "v":
                        nc.vector.tensor_copy(obc, pa[:])
                    else:
                        nc.scalar.copy(obc, pa[:])
                else:
                    pa = p_pa.tile([128, 512], FP32, tag="pa")
                    nc.tensor.matmul(pa[:], lhsT=blk2[:],
                                     rhs=x_bf[:, nk * 512:(nk + 1) * 512],
                                     start=True, stop=True)
                    obc = ob[:, nk * 512:(nk + 1) * 512]
                    if epi_dve[nk % len(epi_dve)] == "v":
                        nc.vector.tensor_copy(obc, pa[:])
                    else:
                        nc.scalar.copy(obc, pa[:])
                rows = slice(idx * 128, (idx + 1) * 128)
                for i in range(out_pieces):
                    if out_gate[i] == nk:
                        c0, c1 = i * opc, (i + 1) * opc
                        if contig:
                            dma_eng(out_eng[i % len(out_eng)]).dma_start(
                                out_h[idx * out_pieces + i], ob[:, c0:c1])
                        else:
                            dma_eng(out_eng[i % len(out_eng)]).dma_start(
                                out_h[rows, c0:c1], ob[:, c0:c1])

            def emit_pipeline():
                xs.clear(); As.clear(); Bs.clear()
                if prefetch:
                    for j in range(min(pdist, n)):
                        f_in(j)
                for s in range(n + 2):
                    pb_, pa_ = s - 2, s - 1
                    pi_ = s + pdist if prefetch else s
                    if 0 <= pb_:
                        b_pre(pb_)
                    if pi_ < n:
                        f_in(pi_)
                    if 0 <= pa_ < n:
                        a_pre(pa_)
                    for q in range(NGR):
                        if b_first and 0 <= pb_:
                            b_chunk(pb_, 2 * q)
                            b_chunk(pb_, 2 * q + 1)
                        if 0 <= pa_ < n:
                            a_group(pa_, q)
                            if q > 0:
                                a_gram(pa_, q - 1)
                        if not b_first and 0 <= pb_:
                            b_chunk(pb_, 2 * q)
                            b_chunk(pb_, 2 * q + 1)
                    if 0 <= pa_ < n:
                        a_gram(pa_, NGR - 1)
                        a_post(pa_)

            if loops:
                with tc.For_i(0, loops, 1):
                    emit_pipeline()
            else:
                emit_pipeline()

    nc.finalize()
    return nc


# Final kernel configuration. Selected by on-hardware For_i-loop differencing
# (see test.py): within one measurement round, steady-state per-body times were
#   v2 (in8, coarse stages)        18.4us
#   v5 in4/out4 single sync queue  17.5us   <-- chosen
#   v5 + gpsimd small ops          22.0us   (real Q7 launch cost >> model)
#   v5 + out DMAs on ACT queue     28.7us   (ACT sequencer DMA config stalls)
# Probes: PE-only 9.4us, drain-only 11.3us, DMA-only(in4o4) 16.8us => the
# kernel is DMA-queue-path bound; fine-grained A/B stage interleave (v5) keeps
# the compute engines fed while the sync-queue DMAs serialize.
# Round 8 (in-round controls): input-DMA prefetch one pipeline step earlier
# (x_bufs=4) fixes head-of-line blocking on the single SP DMA queue — output
# DMAs waiting at the queue head for epilogue sems were stalling the next
# pair's loads queued behind them: 16.3us/body vs 17.9 without prefetch.
# Rounds 11-12: with head-of-line blocking solved by prefetch, fewer output
# DMAs win monotonically (out4 14.9 -> out2 14.7/15.1 -> out1 14.8 in-round
# bests); input granularity must stay at 4 pieces (in2 regressed 1.1us).
BUILD = build_program_v5
BUILD_KWARGS = {"in_pieces": 4, "out_pieces": 1, "prefetch": True,
                "pdist": 2, "x_bufs": 5}


def make_in_maps(x: np.ndarray) -> list[dict]:
    """Shard [B,1,C,T] fp32 x into per-core bf16 inputs.

    In contig mode the host pre-shuffles each pair's rows into
    [pair*in_pieces + piece][128 rows][piece cols] so every device DMA reads
    one fully sequential HBM block.
    """
    xs = np.ascontiguousarray(x[:, 0]).reshape(N_CORES, SPC * C, T)
    xb = xs.astype(ml_dtypes.bfloat16)
    eye = np.eye(128, dtype=ml_dtypes.bfloat16)
    if BUILD_KWARGS.get("contig"):
        ip = BUILD_KWARGS.get("in_pieces", 4)
        ipc = T // ip
        xb = xb.reshape(N_CORES, PAIRS, 128, ip, ipc).transpose(0, 1, 3, 2, 4)
        xb = xb.reshape(N_CORES, PAIRS * ip, 128, ipc)
    return [{"x": np.ascontiguousarray(xb[r]), "ident": eye}
            for r in range(N_CORES)]


def _run(x, w1, b1, w2, b2, gamma, **run_kwargs):
    x = np.asarray(x, dtype=np.float32)
    w1 = np.asarray(w1, dtype=np.float64)
    b1 = np.asarray(b1, dtype=np.float64)
    w2 = np.asarray(w2, dtype=np.float64)
    b2 = np.asarray(b2, dtype=np.float64)
    gamma = np.asarray(gamma, dtype=np.float64)
    assert x.shape == (B, 1, C, T), x.shape

    A = float(w1 @ w2)
    B2c = float(b1 @ w2)
    gam = float(gamma.reshape(-1)[0])

    nc = BUILD(A, B2c, gam, **BUILD_KWARGS)
    in_maps = make_in_maps(x)
    res = run_bass_kernel_spmd(nc, in_maps, list(range(N_CORES)), **run_kwargs)
    out = np.stack([np.asarray(res.results[r]["out"]) for r in range(N_CORES)])
    if BUILD_KWARGS.get("contig"):
        op = BUILD_KWARGS.get("out_pieces", 4)
        opc = T // op
        out = out.reshape(N_CORES, PAIRS, op, 128, opc).transpose(0, 1, 3, 2, 4)
    out = out.astype(np.float32).reshape(B, C, T)[:, None]
    return out, res


def kernel(x, w1, b1, w2, b2, gamma):
    out, _ = _run(x, w1, b1, w2, b2, gamma)
    return out


def make_timed_runner(nc, in_maps):
    """Build a jitted 8-core runner (no donation) for repeat timing.

    Mirrors bass2jax.run_bass_via_pjrt's multi-core path but keeps the jitted
    function so the NEFF can be executed repeatedly with device-resident args.
    """
    import jax
    import numpy as _np
    from jax.sharding import Mesh, PartitionSpec
    from jax.experimental.shard_map import shard_map

    import concourse.mybir as _mybir
    from concourse import bass2jax
    from concourse.bass2jax import _bass_exec_p, install_neuronx_cc_hook

    install_neuronx_cc_hook()
    n_cores = len(in_maps)
    partition_name = nc.partition_id_tensor.name if nc.partition_id_tensor else None

    in_names, out_names, out_avals, zero_outs = [], [], [], []
    for alloc in nc.m.functions[0].allocations:
        if not isinstance(alloc, _mybir.MemoryLocationSet):
            continue
        name = alloc.memorylocations[0].name
        if alloc.kind == "ExternalInput":
            if name != partition_name:
                in_names.append(name)
        elif alloc.kind == "ExternalOutput":
            out_names.append(name)
            shape = tuple(alloc.tensor_shape)
            dtype = _mybir.dt.np(alloc.dtype)
            out_avals.append(jax.core.ShapedArray(shape, dtype))
            zero_outs.append(_np.zeros(shape, dtype))
    n_params = len(in_names)
    in_names = in_names + out_names
    if partition_name is not None:
        in_names.append(partition_name)

    def _exec_once(*args):
        operands = list(args)
        if partition_name is not None:
            operands.append(bass2jax.partition_id_tensor())
        outs = _bass_exec_p.bind(
            *operands,
            out_avals=tuple(out_avals),
            in_names=tuple(in_names),
            out_names=tuple(out_names),
            lowering_input_output_aliases=(),
            sim_require_finite=True,
            sim_require_nnan=True,
            nc=nc,
        )
        return tuple(outs)

    assert len(out_names) == 1

    devices = jax.devices()[:n_cores]
    mesh = Mesh(_np.asarray(devices), ("core",))
    in_specs = (PartitionSpec("core"),) * (n_params + len(out_names))
    out_specs = (PartitionSpec("core"),) * len(out_names)
    fn = jax.jit(
        shard_map(_exec_once, mesh=mesh, in_specs=in_specs, out_specs=out_specs,
                  check_rep=False),
        keep_unused=True,
    )
    concat_in = [
        _np.concatenate([_np.asarray(in_maps[c][nm]) for c in range(n_cores)], axis=0)
        for nm in in_names[:n_params]
    ]
    concat_zeros = [
        _np.zeros((n_cores * z.shape[0], *z.shape[1:]), z.dtype) for z in zero_outs
    ]
    shard = jax.sharding.NamedSharding(mesh, PartitionSpec("core"))
    args = [jax.device_put(a, shard) for a in concat_in + concat_zeros]

    def run():
        o = fn(*args)[0]
        return jax.block_until_ready(o)

    run.fn = fn
    run.args = args
    return run, out_names, out_avals


def timed_run(x, w1, b1, w2, b2, gamma, r1=2, r2=10, reps=15,
              build=None):
    """Measure per-kernel device time via the slope between two NEFFs that
    run the whole kernel body `r1` and `r2` times internally (the constant
    axon RPC overhead cancels in the difference)."""
    import time as _time

    x = np.asarray(x, dtype=np.float32)
    A = float(np.asarray(w1, np.float64) @ np.asarray(w2, np.float64))
    B2c = float(np.asarray(b1, np.float64) @ np.asarray(w2, np.float64))
    gam = float(np.asarray(gamma, np.float64).reshape(-1)[0])
    in_maps = make_in_maps(x)

    t_best = {}
    out_arr = None
    out_avals = None
    if build is None:
        def build(A_, B2_, g_, replicate=1):
            return BUILD(A_, B2_, g_, replicate=replicate, **BUILD_KWARGS)
    for rep in (r1, r2):
        nc = build(A, B2c, gam, replicate=rep)
        run, out_names, out_avals = make_timed_runner(nc, in_maps)
        out_arr = run()  # compile + warmup
        run()
        best = None
        for _ in range(reps):
            t0 = _time.perf_counter_ns()
            run()
            dt = _time.perf_counter_ns() - t0
            best = dt if best is None else min(best, dt)
        t_best[rep] = best

    per_exec_ns = (t_best[r2] - t_best[r1]) / (r2 - r1)
    out = np.asarray(out_arr)
    out = out.reshape(N_CORES, *out_avals[0].shape).reshape(B, C, T)[:, None]
    return out.astype(np.float32), per_exec_ns

